# revision 9
# baseline (speedup 1.0000x reference)
"""Trainium2 Bass kernel for nn_Attention_27358941675773.

Reference computation (per batch b):
    q = x @ Q              [N, H]
    k = x @ K              [N, H]
    V = V_down @ V_up      [L, L]
    v = x @ V              [N, L]
    S = q @ k.T / 256      [N, N]
    out = softmax(S) @ v   [N, L]

Sharding: pure data-parallel over batch B=8 across the 8 NeuronCores
(one batch element per core); small params replicated. No collectives.

Per-core kernel design (N=4096, L=256, H=128):
  - Inputs shipped as fp16 (x transposed to [L, N]); all matmuls run at
    full PE rate. qT [H,N] / kT [H,N] are computed directly transposed so
    scores are built as S_T[m, n] (keys on partitions), no transposes.
  - Value path factored through the rank-H bottleneck:
        out = softmax(S) @ x @ V_down @ V_up
    so the O(N^2) product contracts into H=128 columns.
  - PSUM layout (8 banks): one 3-slot score ring [128, 3, 1024] f32
    (6 banks, manually indexed) + mid accumulator [128, 1024] f32
    (2 banks). exp runs on PAIRS of score tiles via a 3-dim AP over the
    ring (strides +4KB or -8KB), halving the per-instruction overhead on
    the Scalar engine, which paces the whole kernel.
  - Rowsum of exp-scores: a pairwise tree of 2048-wide bf16 adds on the
    Vector engine over the 15 leading pairs (17 ops/block instead of 31
    narrow ones), with the last pair folded separately so the
    post-last-exp chain is only two 1024-wide adds.
  - Partition-axis reduction+broadcast of the rowsum in ONE matmul with
    an all-ones [128,128] stationary operand (fp32r, full PE rate),
    replacing the 8.3us GpSimd PartitionAllReduce.
  - Normalization applied after V_up at the output-copy stage; the
    numerator copy (mid -> SBUF) rides the Vector engine, keeping the
    Scalar engine exclusively on the exp stream.
  - DMA: x arrives in 4 large descriptors on the SP queue; weights in 4
    single-issue strided descriptors on the GpSimd queue - the serialized
    ~0.7us-per-issue descriptor cost no longer delays the first exp.
  - Uniform half-block-lagged schedule as before: per pair-slot the PE
    runs 4 QK matmuls + 4 lagged attention@w matmuls; block 0 uses the
    projections as filler; ~10 junk matmuls warm the PE clock gate.
  - Output stored transposed [L, N] fp16; host un-transposes on gather.
"""

import os
import sys

import numpy as np

for _p in ("/opt/trn_rl_repo",):
    if _p not in sys.path and os.path.isdir(_p):
        sys.path.insert(0, _p)

B, N, L, H = 8, 4096, 256, 128
SCALER = 256.0
NB = 1024           # query-block (free dim of score tiles)
NBH = 512           # half tile (one PSUM bank of fp32)
NT = N // NB        # 4 query blocks
MT = N // 128       # 32 key tiles of 128
NP = 16             # key-tile PAIRS per block
P = 128


def _build():
    import concourse.bass as bass
    import concourse.tile as tile
    from concourse import bacc, bass_isa, mybir
    from contextlib import ExitStack

    f32 = mybir.dt.float32
    f32r = mybir.dt.float32r
    f16 = mybir.dt.float16
    bf16 = mybir.dt.bfloat16
    AF = mybir.ActivationFunctionType

    nc = bacc.Bacc(
        "TRN2", target_bir_lowering=False, debug=False, num_devices=B
    )

    xT_ext = nc.declare_dram_parameter("xT", [L, N], f16, isOutput=False)
    wq_ext = nc.declare_dram_parameter("Wq", [L, H], f16, isOutput=False)
    wk_ext = nc.declare_dram_parameter("Wk", [L, H], f16, isOutput=False)
    vd_ext = nc.declare_dram_parameter("Vd", [L, H], f16, isOutput=False)
    vu_ext = nc.declare_dram_parameter("Vu", [H, L], f16, isOutput=False)
    # output stored transposed [L, N]; host un-transposes at gather
    out_ext = nc.declare_dram_parameter("out", [L, N], f16, isOutput=True)

    with tile.TileContext(nc) as tc, ExitStack() as ctx:
        persist = ctx.enter_context(tc.tile_pool(name="persist", bufs=1))

        ones32f = persist.tile([P, P], f32)
        nc.gpsimd.memset(ones32f[:], 1.0)
        ones32 = persist.tile([P, P], f32r)
        nc.vector.tensor_copy(ones32[:], ones32f[:])
        # touch Exp right away so the ~2.7us ACT table load overlaps the
        # input DMAs instead of delaying the first real exp
        dum = persist.tile([1, 2], f32)
        nc.gpsimd.memset(dum[:], 0.0)
        nc.scalar.activation(dum[:, 1:2], dum[:, 0:1], AF.Exp)
        wrm = persist.tile([P, NBH], bf16, name="wrm")
        nc.vector.memset(wrm[:], 0.0)

        qw16 = persist.tile([P, 2 * H], f16)    # Q   [l_chunk][l_in, h]
        kw16 = persist.tile([P, 2 * H], f16)
        vd16 = persist.tile([P, 2 * H], f16)    # V_down [l_chunk][l_in, h]
        vu16 = persist.tile([P, L], f16)        # V_up   [h, l]
        vu_bf = persist.tile([P, L], bf16)      # V_up as bf16 (out matmul)
        xt16 = [persist.tile([P, N], f16, name=f"xt16_{c}") for c in range(2)]
        qT16 = persist.tile([P, N], f16)        # q.T       [h, n]
        kT16 = persist.tile([P, N], f16)        # k.T       [h, m]
        w_sb = persist.tile([P, MT * H], bf16)  # x@V_down  [m_tile][m_in, h]

        # ---------------- phase A: direct fp16 loads ----------------
        # x s0 chunks first (critical path for the first QK tiles), as
        # two large descriptors; the rest of x as two more. Weights ride
        # the GpSimd queue concurrently, one descriptor per tensor.
        for c in range(2):
            nc.sync.dma_start(
                xt16[c][:, 0:NB], xT_ext[c * P:(c + 1) * P, 0:NB]
            )
        for w_ext, w_sbuf in ((wq_ext, qw16), (wk_ext, kw16), (vd_ext, vd16)):
            nc.gpsimd.dma_start(
                w_sbuf[:].rearrange("p (c h) -> p c h", c=2),
                w_ext[:, :].rearrange("(c p) h -> p c h", c=2),
            )
        nc.gpsimd.dma_start(vu16[:], vu_ext[:, :])
        for c in range(2):
            nc.sync.dma_start(
                xt16[c][:, NB:N], xT_ext[c * P:(c + 1) * P, NB:N]
            )
        nc.vector.tensor_copy(vu_bf[:], vu16[:])

        # ------------- phases B+C: projections fused with attention -------
        with (
            tc.tile_pool(name="ring", bufs=1, space="PSUM") as ring_pool,
            tc.tile_pool(name="mtp", bufs=1, space="PSUM") as mtp,
            tc.tile_pool(name="est", bufs=18) as est_pool,
            tc.tile_pool(name="tree", bufs=3) as tree_pool,
            tc.tile_pool(name="sb_small", bufs=2) as sb_small,
            tc.tile_pool(name="outfin", bufs=4) as outfin_pool,
        ):
            # manual 3-slot score ring: slots i at banks (2i, 2i+1)
            sr = ring_pool.tile([P, 3, NB], f32, name="score_ring")
            gslot = [0]      # global ring-use counter

            def slot():
                i = gslot[0] % 3
                gslot[0] += 1
                return i

            est = {}      # (k, pair) -> bf16 [128, 2048] exp tiles
            mtiles = {}   # k -> psum numerator mid^T [h, n] tile
            mscs = {}     # k -> normalized mid (bf16, SBUF)
            bc = {}       # k -> [128, NB] f32 broadcast 1/rowsum (SBUF)
            u1 = {}       # tree level tiles
            u2 = {}
            u3 = {}
            u4 = {}
            t5p = {}
            t5 = {}

            def proj_qkT_pair(w16, dst, f, on_act=False):
                # projects halves f and f+1 into one ring slot, one copy
                i = slot()
                for half in range(2):
                    ff = f + half
                    for c in range(2):
                        nc.tensor.matmul(
                            sr[:, i, half * NBH:(half + 1) * NBH],
                            w16[:, c * H:(c + 1) * H],
                            xt16[c][:, ff * NBH:(ff + 1) * NBH],
                            start=(c == 0), stop=(c == 1),
                        )
                if on_act:
                    nc.scalar.activation(
                        dst[:, f * NBH:(f + 2) * NBH], sr[:, i, :], AF.Copy
                    )
                else:
                    nc.vector.tensor_copy(
                        dst[:, f * NBH:(f + 2) * NBH], sr[:, i, :]
                    )

            def proj_w_batch(b):
                # w tiles 4b..4b+3 into one ring slot, one copy
                i = slot()
                for j4 in range(4):
                    j = 4 * b + j4
                    for c in range(2):
                        nc.tensor.matmul(
                            sr[:, i, j4 * H:(j4 + 1) * H],
                            xt16[c][:, j * P:(j + 1) * P],
                            vd16[:, c * H:(c + 1) * H],
                            start=(c == 0), stop=(c == 1),
                        )
                nc.vector.tensor_copy(
                    w_sb[:, b * NBH:(b + 1) * NBH], sr[:, i, 0:NBH]
                )

            def qk_exp_pair(k, p):
                i0 = slot()
                i1 = slot()
                for t, i in ((2 * p, i0), (2 * p + 1, i1)):
                    for h in range(2):
                        nc.tensor.matmul(
                            sr[:, i, h * NBH:(h + 1) * NBH],
                            kT16[:, t * P:(t + 1) * P],
                            qT16[:, k * NB + h * NBH: k * NB + (h + 1) * NBH],
                            start=True, stop=True,
                        )
                if i1 == i0 + 1:
                    src = sr[:, i0:i0 + 2, :]
                else:          # (i0, i1) == (2, 0): stride -2 pair
                    src = sr[:, i0::-2, :]
                e = est_pool.tile([P, 2 * NB], bf16, tag="est",
                                  name=f"est_{k}_{p}")
                est[(k, p)] = e
                nc.scalar.activation(e[:], src, AF.Exp, scale=1.0 / SCALER)

            def pv_pair(kk, p, mid):
                for j in (2 * p, 2 * p + 1):
                    ej = est[(kk, j // 2)]
                    off = (j % 2) * NB
                    for h in range(2):
                        nc.tensor.matmul(
                            mid[:, h * NBH:(h + 1) * NBH],
                            w_sb[:, j * H:(j + 1) * H],
                            ej[:, off + h * NBH: off + (h + 1) * NBH],
                            start=(j == 0), stop=(j == MT - 1),
                        )

            def tree_adds(k, p):
                # 2048-wide pairwise tree over pairs 0..14; pair 15 is
                # folded separately at the next block head (short tail)
                if p % 2 == 1 and p <= 13:
                    t = tree_pool.tile([P, 2 * NB], bf16, tag="u1", bufs=2,
                                       name=f"u1_{k}_{p}")
                    nc.vector.tensor_add(t[:], est[(k, p - 1)][:], est[(k, p)][:])
                    u1[(k, p // 2)] = t
                if p in (3, 7, 11):
                    t = tree_pool.tile([P, 2 * NB], bf16, tag="u2", bufs=2,
                                       name=f"u2_{k}_{p}")
                    nc.vector.tensor_add(
                        t[:], u1[(k, p // 2 - 1)][:], u1[(k, p // 2)][:]
                    )
                    u2[(k, p // 4)] = t
                if p == 14:
                    t = tree_pool.tile([P, 2 * NB], bf16, tag="u2", bufs=2,
                                       name=f"u2_{k}_14")
                    nc.vector.tensor_add(t[:], u1[(k, 6)][:], est[(k, 14)][:])
                    u2[(k, 3)] = t
                if p == 7:
                    t = tree_pool.tile([P, 2 * NB], bf16, tag="u3", bufs=2,
                                       name=f"u3_{k}_0")
                    nc.vector.tensor_add(t[:], u2[(k, 0)][:], u2[(k, 1)][:])
                    u3[(k, 0)] = t
                if p == 14:
                    t = tree_pool.tile([P, 2 * NB], bf16, tag="u3", bufs=2,
                                       name=f"u3_{k}_1")
                    nc.vector.tensor_add(t[:], u2[(k, 2)][:], u2[(k, 3)][:])
                    u3[(k, 1)] = t
                    t4 = tree_pool.tile([P, 2 * NB], f32, tag="u4", bufs=1,
                                        name=f"u4_{k}")
                    nc.vector.tensor_add(t4[:], u3[(k, 0)][:], u3[(k, 1)][:])
                    u4[k] = t4
                if p == 15:
                    t = tree_pool.tile([P, NB], f32, tag="t5p", bufs=2,
                                       name=f"t5p_{k}")
                    nc.vector.tensor_add(
                        t[:], u4[k][:, 0:NB], u4[k][:, NB:2 * NB]
                    )
                    t5p[k] = t

            def fold_last(k):
                # sum of pair 15 + partial -> full per-partition rowsum
                f15 = tree_pool.tile([P, NB], f32, tag="f15", bufs=1,
                                     name=f"f15_{k}")
                nc.vector.tensor_add(
                    f15[:], est[(k, 15)][:, 0:NB], est[(k, 15)][:, NB:2 * NB]
                )
                t = tree_pool.tile([P, NB], f32r, tag="t5", bufs=1,
                                   name=f"t5_{k}")
                nc.vector.tensor_add(t[:], t5p[k][:], f15[:])
                t5[k] = t

            def bc_chain(k):
                # partition-axis sum + broadcast in one matmul: all-ones
                # stationary, fp32r at full PE rate; then fast reciprocal
                i = slot()
                for h in range(2):
                    nc.tensor.matmul(
                        sr[:, i, h * NBH:(h + 1) * NBH],
                        ones32[:],
                        t5[k][:, h * NBH:(h + 1) * NBH],
                        start=True, stop=True,
                    )
                bck = sb_small.tile([P, NB], f32, tag="bc", bufs=2,
                                    name=f"bc_{k}")
                nc.vector.reciprocal_approx_fast(bck[:], sr[:, i, :])
                bc[k] = bck

            def norm_mid(k):
                msc = sb_small.tile([P, NB], bf16, tag="msc", bufs=2,
                                    name=f"msc_{k}")
                nc.vector.tensor_copy(msc[:], mtiles[k][:])
                mscs[k] = msc

            def drain_out(k):
                # apply V_up, normalize by 1/rowsum, store transposed (f16)
                for lt in range(2):
                    i = slot()
                    for h in range(2):
                        nc.tensor.matmul(
                            sr[:, i, h * NBH:(h + 1) * NBH],
                            vu_bf[:, lt * P:(lt + 1) * P],
                            mscs[k][:, h * NBH:(h + 1) * NBH],
                            start=True, stop=True,
                        )
                    fin = outfin_pool.tile([P, NB], f16, tag="fin")
                    nc.vector.tensor_mul(fin[:], sr[:, i, :], bc[k][:])
                    nc.gpsimd.dma_start(
                        out_ext[lt * P:(lt + 1) * P, k * NB:(k + 1) * NB],
                        fin[:],
                    )

            # PE warm-up: junk matmuls while the x DMA is in flight
            for _ in range(10):
                i = slot()
                nc.tensor.matmul(
                    sr[:, i, 0:NBH], wrm[:, :P], wrm[:], start=True, stop=True
                )

            # head: first QK tiles need qT/kT half-blocks 0,1 (chunk s0)
            proj_qkT_pair(qw16, qT16, 0, on_act=True)
            proj_qkT_pair(kw16, kT16, 0, on_act=False)

            for k in range(NT):
                for p in range(NP):
                    # PV first in emission order so the PE never
                    # head-blocks behind a QK that waits on the exp ring
                    if k >= 1 and p <= 7:
                        pv_pair(k - 1, 8 + p, mtiles[k - 1])
                    if p >= 9:
                        pv_pair(k, p - 8, mtiles[k])
                    if k == NT - 1 and p >= 12:
                        # last block: pull part of the epilogue forward
                        pv_pair(k, p - 4, mtiles[k])
                    qk_exp_pair(k, p)
                    if p == 8:
                        # after this pair's QK/exp: the first PV of the new
                        # block waits on the mid-copy (DVE) and must not
                        # head-block the PE queue at the boundary
                        mid = mtp.tile([P, NB], f32, tag="mtp",
                                       name=f"mid_{k}")
                        mtiles[k] = mid
                        pv_pair(k, 0, mtiles[k])
                    if k == 0:
                        if p <= 7:
                            proj_w_batch(p)
                        # kT halves 2..7 ahead of their QK consumers
                        if p in (1, 3, 5):
                            proj_qkT_pair(kw16, kT16, p + 1, on_act=False)
                        # qT halves for blocks 1..3
                        if p in (7, 9):
                            proj_qkT_pair(qw16, qT16, p - 5, on_act=False)
                    if k == 1 and p == 1:
                        proj_qkT_pair(qw16, qT16, 6, on_act=False)
                    if k >= 1:
                        if p == 0:
                            fold_last(k - 1)
                        if p == 1:
                            bc_chain(k - 1)
                        if p == 7:
                            norm_mid(k - 1)
                        if p == 10:
                            drain_out(k - 1)
                    tree_adds(k, p)

            # epilogue: block 3 rowsum chain + remaining PV + drain
            k3 = NT - 1
            fold_last(k3)
            bc_chain(k3)
            for p in range(12, 16):
                pv_pair(k3, p, mtiles[k3])
            norm_mid(k3)
            drain_out(k3)

    if not nc.is_finalized():
        nc.finalize()
    return nc


_GRAPH_CACHE = {}


def _get_graph():
    if "nc" not in _GRAPH_CACHE:
        _GRAPH_CACHE["nc"] = _build()
    return _GRAPH_CACHE["nc"]


def run(inputs: dict, trace: bool = False):
    """Run the SPMD kernel on 8 cores. Returns (output, BassKernelResults)."""
    from concourse.bass_utils import run_bass_kernel_spmd

    x = np.asarray(inputs["x"], dtype=np.float32)
    Q = np.asarray(inputs["Q"], dtype=np.float32)[0]
    K = np.asarray(inputs["K"], dtype=np.float32)[0]
    Vd = np.asarray(inputs["V_down"], dtype=np.float32)[0]
    Vu = np.asarray(inputs["V_up"], dtype=np.float32)[0]

    wq = np.ascontiguousarray(Q).astype(np.float16)
    wk = np.ascontiguousarray(K).astype(np.float16)
    vd = np.ascontiguousarray(Vd).astype(np.float16)
    vu = np.ascontiguousarray(Vu).astype(np.float16)

    in_maps = []
    for b in range(B):
        in_maps.append({
            "xT": np.ascontiguousarray(x[b].T).astype(np.float16),
            "Wq": wq,
            "Wk": wk,
            "Vd": vd,
            "Vu": vu,
        })

    nc = _get_graph()
    res = run_bass_kernel_spmd(nc, in_maps, core_ids=list(range(B)), trace=trace)
    # device output is [L, N] per core; un-transpose during the gather
    out = np.stack([np.asarray(res.results[i]["out"]).astype(np.float32).T for i in range(B)])
    return np.ascontiguousarray(out, dtype=np.float32), res


def kernel(**inputs) -> np.ndarray:
    out, _ = run(inputs, trace=False)
    return out
